# revision 4
# baseline (speedup 1.0000x reference)
"""CenterLoss Trainium2 kernel, v2.

loss = mean_i ||x[i] - centers[labels[i]]||^2

The one-hot distmat collapses to a row gather of `centers`. Sharding is
the hint's model-parallel variant: the host sorts batch rows by label and
gives core k the k-th 512-row chunk of the sorted order, so each core's
labels fall in a narrow class window (span ~6.5k << 50000). Each core
receives only an 8192-row slice of `centers` starting at its window base,
and local indices (label - base) fit int16 -- which unlocks the batched
SWDGE `dma_gather` ucode: ONE gather instruction for all 512 rows
(994ns fixed + 0.34ns/row) instead of 4 serialized 128-row indirect DMAs
(~1us fixed EACH, the v1 critical path).

Per core:
  - ACT triggers the 8KB int16 index DMA (single packet)
  - SP triggers the 256KB x DMA (host pre-transposed to [128, 4*128] so
    every partition row is one contiguous 2KB chunk)
  - GpSimd: one dma_gather of 512 x 512B center rows -> [128, 4, 128]
  - DVE: subtract, then fused square+row-accumulate -> acc[128, 1]
  - DVE triggers the 512B acc DMA out; host sums 8x128 partials / 4096
    (the all-reduce-the-mean step).
"""

import os
import sys

import numpy as np

for _p in (
    "/opt/trn_rl_repo",
    "/root/.axon_site/_ro/trn_rl_repo",
    "/root/.axon_site",
    "/root/.axon_site/_ro/pypackages",
):
    if os.path.isdir(_p) and _p not in sys.path:
        sys.path.append(_p)

NCORES = 8
B = 4096
D = 128
C = 50000
P = 128
B_LOC = B // NCORES          # 512 rows per core
NT = B_LOC // P              # 4 row-tiles of 128
CSPAN = 8192                 # per-core centers window (> max sorted-chunk span)

_cached = None


def _build():
    import concourse.bacc as bacc
    import concourse.mybir as mybir
    from concourse.library_config import mlp

    nc = bacc.Bacc(
        "TRN2",
        target_bir_lowering=False,
        debug=False,
        enable_asserts=False,
        num_devices=NCORES,
    )

    # Bass.__init__ unconditionally emits a const-AP pool (4 gpsimd memsets)
    # plus an all-engine barrier; nothing in this kernel reads those consts.
    for blk in nc.main_func.blocks:
        blk.instructions[:] = [
            ins
            for ins in blk.instructions
            if type(ins).__name__
            not in ("InstMemset", "InstDrain", "InstEventSemaphore")
        ]

    x_h = nc.dram_tensor("x", [P, NT * D], mybir.dt.float32, kind="ExternalInput")
    idx_h = nc.dram_tensor("labels", [P, B_LOC // 16], mybir.dt.int16, kind="ExternalInput")
    cen_h = nc.dram_tensor("centers", [CSPAN, D], mybir.dt.float32, kind="ExternalInput")
    out_h = nc.dram_tensor("out", [P, 1], mybir.dt.float32, kind="ExternalOutput")

    with (
        nc.Block(no_gpsimd_drain=True) as block,
        nc.sbuf_tensor("xs", [P, NT * D], mybir.dt.float32) as xs,
        nc.sbuf_tensor("ids", [P, B_LOC // 16], mybir.dt.int16) as ids,
        nc.sbuf_tensor("cs", [P, NT, D], mybir.dt.float32) as cs,
        nc.sbuf_tensor("acc", [P, 1], mybir.dt.float32) as acc,
        nc.semaphore("s_idx") as s_idx,
        nc.semaphore("s_x") as s_x,
        nc.semaphore("s_g") as s_g,
        nc.semaphore("s_c") as s_c,
        nc.semaphore("s_o") as s_o,
    ):
        @block.scalar
        def _(scalar):
            scalar.dma_start(ids[:], idx_h.ap(), single_packet=True).then_inc(s_idx, 16)
            scalar.wait_ge(s_c, 1)
            scalar.dma_start(out_h.ap(), acc[:], single_packet=True).then_inc(s_o, 16)
            scalar.wait_ge(s_o, 16)

        @block.sync
        def _(sync):
            sync.dma_start(xs[:], x_h.ap()).then_inc(s_x, 16)

        @block.gpsimd
        def _(gpsimd):
            gpsimd.load_library(mlp)
            gpsimd.wait_ge(s_idx, 16)
            gpsimd.dma_gather(cs[:], cen_h.ap(), ids[:], B_LOC, B_LOC, D).then_inc(s_g, 16)

        @block.vector
        def _(vector):
            vector.wait_ge(s_g, 16)
            vector.wait_ge(s_x, 16)
            cflat = cs[:].rearrange("p n d -> p (n d)")
            vector.tensor_tensor(
                out=cflat, in0=xs[:], in1=cflat, op=mybir.AluOpType.subtract
            )
            # cflat^2 elementwise, with the free-dim row-sum peeled into acc
            vector.scalar_tensor_tensor(
                out=cflat,
                in0=cflat,
                scalar=1.0,
                in1=cflat,
                op0=mybir.AluOpType.mult,
                op1=mybir.AluOpType.mult,
                accum_out=acc[:],
            ).then_inc(s_c, 1)

    nc.compile()
    return nc


def _get_nc():
    global _cached
    if _cached is None:
        _cached = _build()
    return _cached


def kernel(x, labels, centers, **profile_kwargs):
    from concourse.bass_utils import run_bass_kernel_spmd

    nc = _get_nc()
    x = np.ascontiguousarray(np.asarray(x), dtype=np.float32)
    centers = np.ascontiguousarray(np.asarray(centers), dtype=np.float32)
    labels64 = np.asarray(labels).astype(np.int64)

    order = np.argsort(labels64, kind="stable")
    in_maps = []
    for k in range(NCORES):
        sel = order[k * B_LOC : (k + 1) * B_LOC]
        lab = labels64[sel]
        base = int(min(int(lab[0]), C - CSPAN))
        span = int(lab[-1]) - base
        assert 0 <= span < CSPAN, f"core {k}: class window span {span} >= {CSPAN}"
        idx16 = (lab - base).astype(np.int16)
        # dma_gather idx layout: index j lives at partition j%16, column
        # j//16, replicated across each 16-partition group
        blk = idx16.reshape(B_LOC // 16, 16).T
        idxp = np.ascontiguousarray(np.tile(blk, (P // 16, 1)))
        # x packed so partition p, tile n holds sorted-chunk row n*128+p
        # (matching dma_gather's dst[i%128, i//128] placement)
        xk = np.ascontiguousarray(
            x[sel].reshape(NT, P, D).transpose(1, 0, 2).reshape(P, NT * D)
        )
        in_maps.append(
            {
                "x": xk,
                "labels": idxp,
                "centers": np.ascontiguousarray(centers[base : base + CSPAN]),
            }
        )

    r = run_bass_kernel_spmd(nc, in_maps, core_ids=list(range(NCORES)), **profile_kwargs)
    # out[p] on core k is sum_n ||x - c||^2 over chunk rows n*128+p; the
    # global mean over all 4096 rows is the host-side all-reduce
    total = sum(float(m["out"].sum(dtype=np.float64)) for m in r.results)
    result = np.array(total / B, dtype=np.float32)
    if profile_kwargs:
        return result, r
    return result


# revision 5
# speedup vs baseline: 1.8619x; 1.8619x over previous
"""CenterLoss Trainium2 kernel, v3.

loss = mean_i ||x[i] - centers[labels[i]]||^2

The one-hot distmat collapses to a row gather of `centers`; data-parallel
over 8 cores, 512 batch rows each, centers replicated. The gather uses 4
native INDIRECT1D SWDGE calls (128 rows each -- one index per partition
per call is the HW limit). Measured HW costs that shaped this kernel:
  - ~6us fixed NEFF preamble (engine barriers + TENSOR_LOAD), untouchable
  - ~2us HWDGE trigger->semaphore latency on the tiny label DMA
  - ~1.15us per INDIRECT1D (994ns fixed + per-descriptor), serialized on
    the Pool sequencer; the batched DMAGatherAnt ucode alternative costs
    the same per descriptor PLUS a ~6.5us one-shot library IRAM load
  - DVE ~290ns per [128,128] op; square+row-sum fuse into one
    InstTensorScalarPtr with accum_out

Per core:
  - ACT triggers the 2KB label DMA (single packet, first instruction out
    of the preamble), SP triggers the 256KB x DMA (host pre-transposed to
    [128, 4*128] so each partition row is one contiguous 2KB chunk)
  - GpSimd: 4x indirect-DMA gathers, pipelined with DVE compute
  - DVE per tile: subtract, then fused square+row-accumulate -> acc[:, i]
  - SP: 2KB acc DMA out; host sums 8x512 partials / 4096 (the
    "all-reduce the mean loss" step from the sharding hint)
"""

import os
import sys

import numpy as np

for _p in (
    "/opt/trn_rl_repo",
    "/root/.axon_site/_ro/trn_rl_repo",
    "/root/.axon_site",
    "/root/.axon_site/_ro/pypackages",
):
    if os.path.isdir(_p) and _p not in sys.path:
        sys.path.append(_p)

NCORES = 8
B = 4096
D = 128
C = 50000
P = 128
B_LOC = B // NCORES          # 512 rows per core
NT = B_LOC // P              # 4 row-tiles of 128

_cached = None


def _build():
    import concourse.bacc as bacc
    import concourse.bass as bass
    import concourse.mybir as mybir

    nc = bacc.Bacc(
        "TRN2",
        target_bir_lowering=False,
        debug=False,
        enable_asserts=False,
        num_devices=NCORES,
    )

    # Bass.__init__ unconditionally emits a const-AP pool (4 gpsimd memsets)
    # plus an all-engine barrier; nothing in this kernel reads those consts.
    for blk in nc.main_func.blocks:
        blk.instructions[:] = [
            ins
            for ins in blk.instructions
            if type(ins).__name__
            not in ("InstMemset", "InstDrain", "InstEventSemaphore")
        ]

    x_h = nc.dram_tensor("x", [P, NT * D], mybir.dt.float32, kind="ExternalInput")
    idx_h = nc.dram_tensor("labels", [P, NT], mybir.dt.int32, kind="ExternalInput")
    cen_h = nc.dram_tensor("centers", [C, D], mybir.dt.float32, kind="ExternalInput")
    out_h = nc.dram_tensor("out", [P, NT], mybir.dt.float32, kind="ExternalOutput")

    with (
        nc.Block(no_gpsimd_drain=True) as block,
        nc.sbuf_tensor("xs", [P, NT, D], mybir.dt.float32) as xs,
        nc.sbuf_tensor("ids", [P, NT], mybir.dt.int32) as ids,
        nc.sbuf_tensor("cs", [P, NT, D], mybir.dt.float32) as cs,
        nc.sbuf_tensor("acc", [P, NT], mybir.dt.float32) as acc,
        nc.semaphore("s_idx") as s_idx,
        nc.semaphore("s_x") as s_x,
        nc.semaphore("s_g") as s_g,
        nc.semaphore("s_c") as s_c,
        nc.semaphore("s_o") as s_o,
    ):
        @block.scalar
        def _(scalar):
            scalar.dma_start(ids[:], idx_h.ap(), single_packet=True).then_inc(s_idx, 16)

        @block.sync
        def _(sync):
            sync.dma_start(
                xs[:].rearrange("p n d -> p (n d)"), x_h.ap()
            ).then_inc(s_x, 16)
            sync.wait_ge(s_c, NT)
            sync.dma_start(out_h.ap(), acc[:], single_packet=True).then_inc(s_o, 16)
            sync.wait_ge(s_o, 16)

        @block.gpsimd
        def _(gpsimd):
            gpsimd.wait_ge(s_idx, 16)
            for i in range(NT):
                gpsimd.indirect_dma_start(
                    out=cs[:, i],
                    out_offset=None,
                    in_=cen_h.ap(),
                    in_offset=bass.IndirectOffsetOnAxis(ap=ids[:, i : i + 1], axis=0),
                ).then_inc(s_g, 16)

        @block.vector
        def _(vector):
            vector.wait_ge(s_x, 16)
            for i in range(NT):
                vector.wait_ge(s_g, 16 * (i + 1))
                vector.tensor_tensor(
                    out=cs[:, i],
                    in0=xs[:, i],
                    in1=cs[:, i],
                    op=mybir.AluOpType.subtract,
                )
                # cs^2 elementwise with the free-dim row-sum peeled into acc
                vector.scalar_tensor_tensor(
                    out=cs[:, i],
                    in0=cs[:, i],
                    scalar=1.0,
                    in1=cs[:, i],
                    op0=mybir.AluOpType.mult,
                    op1=mybir.AluOpType.mult,
                    accum_out=acc[:, i : i + 1],
                ).then_inc(s_c, 1)

    nc.compile()
    return nc


def _get_nc():
    global _cached
    if _cached is None:
        _cached = _build()
    return _cached


def kernel(x, labels, centers, **profile_kwargs):
    from concourse.bass_utils import run_bass_kernel_spmd

    nc = _get_nc()
    x = np.ascontiguousarray(np.asarray(x), dtype=np.float32)
    centers = np.ascontiguousarray(np.asarray(centers), dtype=np.float32)
    labels32 = np.asarray(labels).astype(np.int32)

    in_maps = []
    for k in range(NCORES):
        # labels packed so partition p, column n holds the label of row n*P+p
        ls = np.ascontiguousarray(
            labels32[k * B_LOC : (k + 1) * B_LOC].reshape(NT, P).T
        )
        # x packed so partition p, tile n holds batch row n*P+p (contiguous
        # 2KB per partition row -> 128 DMA descriptors instead of 512)
        xk = np.ascontiguousarray(
            x[k * B_LOC : (k + 1) * B_LOC].reshape(NT, P, D).transpose(1, 0, 2).reshape(P, NT * D)
        )
        in_maps.append({"x": xk, "labels": ls, "centers": centers})

    r = run_bass_kernel_spmd(nc, in_maps, core_ids=list(range(NCORES)), **profile_kwargs)
    # out[p, n] on core k is the squared distance row-sum of batch row
    # k*512 + n*128 + p; the mean over all rows is the host-side all-reduce
    total = sum(float(m["out"].sum(dtype=np.float64)) for m in r.results)
    result = np.array(total / B, dtype=np.float32)
    if profile_kwargs:
        return result, r
    return result
